# revision 1
# baseline (speedup 1.0000x reference)
"""Trainium2 Bass kernel for causal multi-head attention (nn_MultiHeadAttention).

Full-input contract: kernel(**inputs) takes the complete tensors
(x [4,2048,1024] f32, Wq/Wk/Wv/Wp [1024,1024], bq/bk/bv/bp [1024]) and
returns the full output [4,2048,1024] f32.

Sharding: 8 cores = 4 batches x 2 head-groups (8 heads / 512 dims each).
Each core computes its head-group's attention output projected through its
row-slice of Wp; the host sums the two partial projections per batch and
adds (bv @ Wp + bp) (exact because softmax rows sum to 1, so the bv term
factors out of the attention).

Host prep (layout only): x is cast to bf16 and transposed to feature-major
xt [1024, 2048]; weight slices are cast to bf16. Device does pure compute:
  V  = x @ Wv  [2048, 512] stored with a per-head ones column (V_aug) so the
       attention matmul also produces the softmax denominator.
  QT = (Wq^T @ xT)/8 + bq/8, KT = Wk^T @ xT + bk   (both [512, 2048], bf16)
  Per head: scoresT[k, q] = KT_h-block^T @ QT_h (causal: only q >= k blocks),
  exp on ScalarE (no max-subtraction needed; |scores| < ~6 by construction),
  diagonal-block triangular mask, outT_aug[d+1, q] accumulated over k-tiles
  in two 1024-column halves (PSUM pressure), normalized via GpSimd
  partition-broadcast of the reciprocal ones-row, giving attnoutT [e, q].
  Final: out_partial[q, :] = attnoutT^T @ Wp_slice, all in bf16 matmuls with
  fp32 PSUM accumulation.
"""
import sys

sys.path.insert(0, "/opt/trn_rl_repo")

import numpy as np
import ml_dtypes

import concourse.bass as bass
import concourse.mybir as mybir
import concourse.tile as tile
from concourse import bacc
from concourse import bass_utils

N_CORES = 8
T = 2048          # tokens per batch
E = 1024          # model dim
D = 512           # head dims per core (8 heads x 64)
H = 8             # heads per core
DH = 64           # head dim
P = 128
FT = E // P       # 8 feature k-tiles
DT = D // P       # 4 local d-tiles
TT = T // P       # 16 token tiles
F32 = mybir.dt.float32
BF16 = mybir.dt.bfloat16
Alu = mybir.AluOpType
Act = mybir.ActivationFunctionType


def _build_program():
    nc = bacc.Bacc(
        "TRN2",
        target_bir_lowering=False,
        debug=False,
        enable_asserts=False,
        num_devices=N_CORES,
    )
    xt_d = nc.dram_tensor("xt", [E, T], BF16, kind="ExternalInput").ap()
    wq_d = nc.dram_tensor("wq", [E, D], BF16, kind="ExternalInput").ap()
    wk_d = nc.dram_tensor("wk", [E, D], BF16, kind="ExternalInput").ap()
    wv_d = nc.dram_tensor("wv", [E, D], BF16, kind="ExternalInput").ap()
    wp_d = nc.dram_tensor("wp", [D, E], BF16, kind="ExternalInput").ap()
    bq8_d = nc.dram_tensor("bq8", [P, DT], F32, kind="ExternalInput").ap()
    bk_d = nc.dram_tensor("bk", [P, DT], F32, kind="ExternalInput").ap()
    tri_d = nc.dram_tensor("tri", [P, P], BF16, kind="ExternalInput").ap()
    out_d = nc.dram_tensor("out", [T, E], F32, kind="ExternalOutput").ap()

    with tile.TileContext(nc) as tc:
        _kernel(tc, xt_d, wq_d, wk_d, wv_d, wp_d, bq8_d, bk_d, tri_d, out_d)
    nc.compile()
    return nc


def _kernel(tc, xt_d, wq_d, wk_d, wv_d, wp_d, bq8_d, bk_d, tri_d, out_d):
    nc = tc.nc
    from contextlib import ExitStack

    with ExitStack() as ctx:
        consts = ctx.enter_context(tc.tile_pool(name="consts", bufs=1))
        wpool = ctx.enter_context(tc.tile_pool(name="wpool", bufs=1))
        big = ctx.enter_context(tc.tile_pool(name="big", bufs=1))
        att = ctx.enter_context(tc.tile_pool(name="att", bufs=8))
        norm = ctx.enter_context(tc.tile_pool(name="norm", bufs=4))
        ostage = ctx.enter_context(tc.tile_pool(name="ostage", bufs=3))
        ps_sc = ctx.enter_context(tc.tile_pool(name="ps_sc", bufs=2, space="PSUM"))
        ps_mm = ctx.enter_context(tc.tile_pool(name="ps_mm", bufs=2, space="PSUM"))
        ps_out = ctx.enter_context(tc.tile_pool(name="ps_out", bufs=2, space="PSUM"))

        # ---- constants ----
        tri = consts.tile([P, P], BF16)
        nc.scalar.dma_start(out=tri, in_=tri_d)
        bq8 = consts.tile([P, DT], F32)
        nc.scalar.dma_start(out=bq8, in_=bq8_d)
        bk = consts.tile([P, DT], F32)
        nc.scalar.dma_start(out=bk, in_=bk_d)

        # ---- direct bf16 loads ----
        wq_b = wpool.tile([P, FT, D], BF16, tag="wq")
        wk_b = wpool.tile([P, FT, D], BF16, tag="wk")
        wv_b = wpool.tile([P, FT, D], BF16, tag="wv")
        wp_b = wpool.tile([P, DT, E], BF16, tag="wp")
        xT = big.tile([P, FT, T], BF16, tag="xT")
        xt_r = xt_d.rearrange("(ft p) t -> p ft t", p=P)
        wv_r = wv_d.rearrange("(ft p) d -> p ft d", p=P)
        # order: V(tt0)'s minimal gating set first (wv + x in ft halves),
        # then wq/wk, the rest of x, wp
        nc.sync.dma_start(out=wv_b[:, 0:4, :], in_=wv_r[:, 0:4, :])
        nc.sync.dma_start(out=xT[:, 0:4, 0:512], in_=xt_r[:, 0:4, 0:512])
        nc.sync.dma_start(out=wv_b[:, 4:8, :], in_=wv_r[:, 4:8, :])
        nc.sync.dma_start(out=xT[:, 4:8, 0:512], in_=xt_r[:, 4:8, 0:512])
        nc.sync.dma_start(out=wq_b, in_=wq_d.rearrange("(ft p) d -> p ft d", p=P))
        nc.sync.dma_start(out=wk_b, in_=wk_d.rearrange("(ft p) d -> p ft d", p=P))
        for tc_i in range(1, 4):
            nc.sync.dma_start(
                out=xT[:, :, tc_i * 512 : (tc_i + 1) * 512],
                in_=xt_r[:, :, tc_i * 512 : (tc_i + 1) * 512],
            )
        nc.sync.dma_start(out=wp_b, in_=wp_d.rearrange("(et p) e -> p et e", p=P))

        qt8 = big.tile([P, DT, T], BF16, tag="qt8")
        kt8 = big.tile([P, DT, T], BF16, tag="kt8")
        vaug = big.tile([P, TT, H * (DH + 1)], BF16, tag="vaug")
        aoutT = big.tile([P, DT, T], BF16, tag="aoutT")

        # ---- V with per-head ones column: [128, tt, 8*65] bf16 ----
        nc.vector.memset(
            vaug.rearrange("p tt (h x) -> p tt h x", x=DH + 1)[:, :, :, DH : DH + 1],
            1.0,
        )
        for tt in range(TT):
            va = vaug[:, tt, :].rearrange("p (h x) -> p h x", x=DH + 1)
            pv = ps_mm.tile([P, 512], F32, tag="mm")
            for ft in range(FT):
                nc.tensor.matmul(
                    pv,
                    lhsT=xT[:, ft, tt * P : (tt + 1) * P],
                    rhs=wv_b[:, ft, :],
                    start=(ft == 0),
                    stop=(ft == FT - 1),
                )
            nc.scalar.activation(
                out=va[:, :, 0:DH],
                in_=pv.rearrange("p (h d) -> p h d", d=DH),
                func=Act.Copy,
            )

        # ---- per d-tile: QT, KT, then 2 heads of attention ----
        pending = []
        for dt_i in range(DT):
            for dst, w_sb, bias, scale in (
                (qt8, wq_b, bq8, 0.125),
                (kt8, wk_b, bk, None),
            ):
                for c in range(4):
                    pq = ps_mm.tile([P, 512], F32, tag="mm")
                    for ft in range(FT):
                        nc.tensor.matmul(
                            pq,
                            lhsT=w_sb[:, ft, dt_i * P : (dt_i + 1) * P],
                            rhs=xT[:, ft, c * 512 : (c + 1) * 512],
                            start=(ft == 0),
                            stop=(ft == FT - 1),
                        )
                    dslice = dst[:, dt_i, c * 512 : (c + 1) * 512]
                    if scale is None:
                        nc.vector.tensor_scalar_add(
                            dslice, pq, bias[:, dt_i : dt_i + 1]
                        )
                    else:
                        nc.vector.tensor_scalar(
                            dslice, pq, scale, bias[:, dt_i : dt_i + 1],
                            op0=Alu.mult, op1=Alu.add,
                        )
            for h in (2 * dt_i, 2 * dt_i + 1):
                _head(tc, h, qt8, kt8, vaug, aoutT, tri, ps_sc, ps_out, att,
                      norm, pending)

        for fn, args in pending:
            fn(*args)

        # ---- output projection ----
        for qt in range(TT):
            ot = ostage.tile([P, E], F32, tag="ot")
            for oc in range(2):
                pp = ps_mm.tile([P, 512], F32, tag="mm")
                for et in range(DT):
                    nc.tensor.matmul(
                        pp,
                        lhsT=aoutT[:, et, qt * P : (qt + 1) * P],
                        rhs=wp_b[:, et, oc * 512 : (oc + 1) * 512],
                        start=(et == 0),
                        stop=(et == DT - 1),
                    )
                nc.scalar.activation(
                    out=ot[:, oc * 512 : (oc + 1) * 512], in_=pp, func=Act.Copy
                )
            nc.sync.dma_start(out=out_d[qt * P : (qt + 1) * P, :], in_=ot)


def _head(tc, h, qt8, kt8, vaug, aoutT, tri, ps_sc, ps_out, att, norm,
          pending):
    """Attention for one head, in two 1024-column q-halves.

    outT accumulates in per-512-column-chunk PSUM tiles; each chunk's
    accumulation (and its softmax denominator row) completes at
    kt == min(4c+3, 15), at which point it is normalized and released.
    """
    nc = tc.nc
    p0 = DH * (h % 2)
    dt_i = h // 2

    def normalize(c, outp):
        recip = norm.tile([1, 512], F32, tag="recip")
        nc.vector.reciprocal(recip, outp[DH : DH + 1, :])
        rb = norm.tile([DH, 512], F32, tag="rb")
        nc.gpsimd.partition_broadcast(rb, recip[0:1, :], channels=DH)
        nc.vector.tensor_tensor(
            aoutT[p0 : p0 + DH, dt_i, c * 512 : (c + 1) * 512],
            outp[0:DH, :],
            rb,
            op=Alu.mult,
        )

    for h2 in range(2):
        qbase = 1024 * h2
        outp = {}
        for c in (2 * h2, 2 * h2 + 1):
            outp[c] = ps_out.tile([DH + 1, 512], F32, tag="outT", name=f"outp{c}")

        def emit_out(kt, expT, q0, c_lo, base, outp=outp, h2=h2):
            # outT accumulation + chunk-completion normalize for one k-tile
            # (outp/h2 bound at definition: the pending queue outlives the half)
            va = vaug[:, kt, :].rearrange("p (h x) -> p h x", x=DH + 1)
            for c in range(c_lo, 2 * h2 + 2):
                cs = max(512 * c, q0)
                ce = 512 * (c + 1)
                nc.tensor.matmul(
                    outp[c][:, cs - 512 * c : ce - 512 * c],
                    lhsT=va[:, h, :],
                    rhs=expT[:, cs - base : ce - base],
                    start=(kt == 0),
                    stop=(kt == min(4 * c + 3, TT - 1)),
                )
            for c in range(c_lo, 2 * h2 + 2):
                if kt == min(4 * c + 3, TT - 1):
                    normalize(c, outp[c])

        # software-pipelined emission: outT for k-tile kt is emitted after
        # the scores/exp of kt+2 (the queue persists across halves and heads,
        # flushed by the caller before the projection)
        for kt in range(8 * (h2 + 1)):
            q0 = max(P * kt, qbase)
            c_lo = max(kt // 4, 2 * h2)
            base = 512 * c_lo  # tile column 0 <-> global q column `base`
            expT = att.tile([P, 1024], BF16, tag="expT")
            sp = ps_sc.tile([P, 1024], F32, tag="sc")
            for c in range(c_lo, 2 * h2 + 2):
                cs = max(512 * c, q0)
                ce = 512 * (c + 1)
                nc.tensor.matmul(
                    sp[:, cs - base : ce - base],
                    lhsT=kt8[p0 : p0 + DH, dt_i, kt * P : (kt + 1) * P],
                    rhs=qt8[p0 : p0 + DH, dt_i, cs:ce],
                    start=True,
                    stop=True,
                )
            nc.scalar.activation(
                out=expT[:, q0 - base : qbase + 1024 - base],
                in_=sp[:, q0 - base : qbase + 1024 - base],
                func=Act.Exp,
            )
            if kt // 8 == h2:
                # diagonal block: zero where q < k
                nc.vector.tensor_tensor(
                    expT[:, q0 - base : q0 - base + P],
                    expT[:, q0 - base : q0 - base + P],
                    tri,
                    op=Alu.mult,
                )
            pending.append((emit_out, (kt, expT, q0, c_lo, base)))
            if len(pending) > 4:
                fn, args = pending.pop(0)
                fn(*args)


_CACHED_NC = None


def _get_nc():
    global _CACHED_NC
    if _CACHED_NC is None:
        _CACHED_NC = _build_program()
    return _CACHED_NC


def make_in_maps(x, Wq, bq, Wk, bk, Wv, bv, Wp, bp):
    bf = ml_dtypes.bfloat16
    x = np.asarray(x, dtype=np.float32)
    tri = np.ascontiguousarray(np.triu(np.ones((P, P), np.float32)).astype(bf))
    in_maps = []
    wq_f = np.asarray(Wq, dtype=np.float32).astype(bf)
    wk_f = np.asarray(Wk, dtype=np.float32).astype(bf)
    wv_f = np.asarray(Wv, dtype=np.float32).astype(bf)
    wp_f = np.asarray(Wp, dtype=np.float32).astype(bf)
    for core in range(N_CORES):
        n, g = core // 2, core % 2
        sl = slice(g * D, (g + 1) * D)
        bq8 = (np.asarray(bq[sl], dtype=np.float32) / 8.0).reshape(DT, P).T
        bkc = np.asarray(bk[sl], dtype=np.float32).reshape(DT, P).T
        in_maps.append(
            {
                "xt": np.ascontiguousarray(x[n].T.astype(bf)),
                "wq": np.ascontiguousarray(wq_f[:, sl]),
                "wk": np.ascontiguousarray(wk_f[:, sl]),
                "wv": np.ascontiguousarray(wv_f[:, sl]),
                "wp": np.ascontiguousarray(wp_f[sl, :]),
                "bq8": np.ascontiguousarray(bq8),
                "bk": np.ascontiguousarray(bkc),
                "tri": tri,
            }
        )
    return in_maps


def assemble_output(results, Wv_b, Wp, bp, bv):
    corr = (np.asarray(bv, dtype=np.float32) @ np.asarray(Wp, dtype=np.float32)) + \
        np.asarray(bp, dtype=np.float32)
    out = np.empty((4, T, E), np.float32)
    for n in range(4):
        out[n] = results[2 * n]["out"] + results[2 * n + 1]["out"] + corr
    return out


def kernel(x, Wq, bq, Wk, bk, Wv, bv, Wp, bp):
    nc = _get_nc()
    in_maps = make_in_maps(x, Wq, bq, Wk, bk, Wv, bv, Wp, bp)
    res = bass_utils.run_bass_kernel_spmd(nc, in_maps, core_ids=list(range(N_CORES)))
    return assemble_output(res.results, Wv, Wp, bp, bv)

